# revision 3
# baseline (speedup 1.0000x reference)
"""Trainium2 Bass kernel for nn_RNN_9363028705408.

ReLU RNN: BATCH=64, SEQ=2048, IN_DIM=512, HID=512.
  gate = X @ W_ih.T + b_ih + b_hh            (GEMM on PE, 3-pass bf16 hi/lo)
  h_t  = relu(gate_t + h_{t-1} @ W_hh.T)     (2048 sequential steps)
Returns (outputs [64,2048,512] f32, h_n [1,64,512] f32).

Data-parallel across 8 cores (8 batch rows each).  Recurrence state is
kept transposed (hidden on partitions): per step 16 matmuls
psum[:, 8q:8q+8] += Whh_tile(k,q) @ hT[:, 8k:8k+8] with bf16 stationary
weights and a bf16 mirror of the fp32 state as the moving operand, fp32
PSUM.  DVE computes s = psum + gate (fp32) and hT_bf16 = relu(s) (the
critical path into the next step); ScalarE writes relu(s) to the fp32
output buffer off the critical path.  The gate GEMM is computed on-chip
from host-pretransposed bf16 hi/lo X^T (X = X_hi + X_lo) with 3 passes
(Xh*Wh + Xl*Wh + Xh*Wl) for near-fp32 gate accuracy.  Host side does all
layout packing (shard/transpose/cast) and the final unpack.
"""
import sys
if '/opt/trn_rl_repo' not in sys.path:
    sys.path.insert(0, '/opt/trn_rl_repo')

from contextlib import ExitStack

import numpy as np
import ml_dtypes

import concourse.bacc as bacc
import concourse.mybir as mybir
import concourse.tile as tile
from concourse.bass_utils import run_bass_kernel_spmd

BATCH, SEQ, IN_DIM, HID = 64, 2048, 512, 512
NCORES = 8
BPC = BATCH // NCORES      # 8 batch rows per core
NQ = HID // 128            # 4 hidden tiles
NK = IN_DIM // 128         # 4 input-dim tiles
SW = NQ * BPC              # 32 = free width of one timestep of hT
TC = 256                   # timesteps per chunk
NCH = SEQ // TC
ROWS_C = TC * BPC          # 2048 rows of X^T consumed per chunk
GMM_N = 512                # moving free size per gate matmul
F32, BF16 = mybir.dt.float32, mybir.dt.bfloat16
BF = ml_dtypes.bfloat16


def build_nc(repeat=1, nsplit=2, gate3=True):
    nc = bacc.Bacc(None, target_bir_lowering=False, debug=False)
    nxt = 2 if gate3 else 1  # hi(+lo) X^T planes
    xt_d = nc.dram_tensor("xt", [128, nxt * NK * SEQ * BPC], BF16, kind="ExternalInput")
    nw = 2 if gate3 else 1   # hi(+lo) W_ih planes
    wih_d = nc.dram_tensor("wih", [128, nw * NK * NQ * 128], BF16, kind="ExternalInput")
    whh_d = nc.dram_tensor("whh", [128, NQ * NQ * 128], BF16, kind="ExternalInput")
    bias_d = nc.dram_tensor("bias", [128, NQ], F32, kind="ExternalInput")
    h0_d = nc.dram_tensor("h0t", [128, SW], BF16, kind="ExternalInput")
    out_d = nc.dram_tensor("outc", [128, SEQ * SW], F32, kind="ExternalOutput")

    with tile.TileContext(nc) as tc, ExitStack() as ctx:
        consts = ctx.enter_context(tc.tile_pool(name="consts", bufs=1))
        xpool = ctx.enter_context(tc.tile_pool(name="x", bufs=2))
        gpool = ctx.enter_context(tc.tile_pool(name="g", bufs=2))
        opool = ctx.enter_context(tc.tile_pool(name="o", bufs=2))
        hpool = ctx.enter_context(tc.tile_pool(name="h", bufs=3))
        spool = ctx.enter_context(tc.tile_pool(name="s", bufs=3))
        pspool = ctx.enter_context(tc.tile_pool(name="ps", bufs=2, space="PSUM"))
        gpspool = ctx.enter_context(tc.tile_pool(name="gps", bufs=2, space="PSUM"))

        wih = consts.tile([128, nw * NK * NQ * 128], BF16)
        nc.sync.dma_start(wih[:], wih_d[:])
        whh = consts.tile([128, NQ * NQ * 128], BF16)
        nc.sync.dma_start(whh[:], whh_d[:])
        bias = consts.tile([128, NQ], F32)
        nc.sync.dma_start(bias[:], bias_d[:])
        h0t = consts.tile([128, SW], BF16)
        nc.sync.dma_start(h0t[:], h0_d[:])

        rep_cm = tc.For_i(0, repeat, 1) if repeat > 1 else None
        if rep_cm is not None:
            rep_cm.__enter__()

        WIH_PLANE = NK * NQ * 128
        XT_PLANE = NK * SEQ * BPC

        prev_hts = None
        for c in range(NCH):
            # ---- load X^T chunk: per plane, 4 k-tiles [128, ROWS_C] bf16 ----
            xts = []
            for pl in range(nxt):
                for k in range(NK):
                    xt = xpool.tile([128, ROWS_C], BF16, tag=f"xt{pl}_{k}")
                    nc.sync.dma_start(
                        xt[:], xt_d[:, pl * XT_PLANE + k * SEQ * BPC + c * ROWS_C:
                                     pl * XT_PLANE + k * SEQ * BPC + (c + 1) * ROWS_C])
                    xts.append(xt)

            # ---- gate GEMM for this chunk -> gT [128, (t, q, b)] f32 ----
            # passes: (Xh, Wh), (Xl, Wh), (Xh, Wl)
            passes = [(0, 0)] if not gate3 else [(0, 0), (1, 0), (0, 1)]
            g = gpool.tile([128, TC * SW], F32)
            g4 = g[:].rearrange("p (t x) -> p t x", x=SW)
            for q in range(NQ):
                for rc in range(ROWS_C // GMM_N):
                    gps = gpspool.tile([128, GMM_N], F32)
                    n = 0
                    nmm = len(passes) * NK
                    for (xpl, wpl) in passes:
                        for k in range(NK):
                            nc.tensor.matmul(
                                gps[:],
                                wih[:, wpl * WIH_PLANE + (k * NQ + q) * 128:
                                       wpl * WIH_PLANE + (k * NQ + q + 1) * 128],
                                xts[xpl * NK + k][:, rc * GMM_N:(rc + 1) * GMM_N],
                                start=(n == 0), stop=(n == nmm - 1))
                            n += 1
                    tpc = GMM_N // BPC  # timesteps covered by this psum (64)
                    dst = g4[:, rc * tpc:(rc + 1) * tpc, 8 * q:8 * q + 8]
                    src = gps[:].rearrange("p (t b) -> p t b", b=BPC)
                    nc.vector.tensor_scalar_add(dst, src, bias[:, q:q + 1])

            # ---- recurrence over this chunk ----
            oc = opool.tile([128, TC * SW], F32)
            for t in range(TC):
                hts = []
                ps = pspool.tile([128, SW], F32)
                n = 0
                src_hts = prev_hts if t == 0 else hts_prev_step
                for k in range(NQ):
                    if src_hts is None:
                        hp = h0t[:, 8 * k:8 * k + 8]
                    else:
                        kw = NQ // len(src_hts)
                        hp = src_hts[k // kw][:, 8 * (k % kw):8 * (k % kw) + 8]
                    for q in range(NQ):
                        nc.tensor.matmul(
                            ps[:, 8 * q:8 * q + 8],
                            whh[:, (k * NQ + q) * 128:(k * NQ + q + 1) * 128],
                            hp,
                            start=(n == 0), stop=(n == 15),
                            skip_group_check=(n != 0))
                        n += 1
                w = SW // nsplit
                for s in range(nsplit):
                    sl = slice(t * SW + s * w, t * SW + (s + 1) * w)
                    stile = spool.tile([128, w], F32, tag=f"s{s}")
                    # s = psum + gate (fp32)
                    nc.vector.tensor_add(stile[:], ps[:, s * w:(s + 1) * w], g[:, sl])
                    # critical path: bf16 relu for next step's matmuls
                    ht = hpool.tile([128, w], BF16, tag=f"h{s}")
                    nc.vector.tensor_scalar_max(ht[:], stile[:], 0.0)
                    hts.append(ht)
                    # off-path: fp32 relu into the output buffer (ScalarE)
                    nc.scalar.activation(oc[:, sl], stile[:],
                                         mybir.ActivationFunctionType.Relu)
                hts_prev_step = hts
            nc.sync.dma_start(out_d[:, c * TC * SW:(c + 1) * TC * SW], oc[:])
            prev_hts = hts_prev_step

        if rep_cm is not None:
            rep_cm.__exit__(None, None, None)
    nc.compile()
    return nc


# ---------------- host-side packing ----------------

def _pack_xt(x_core, gate3=True):
    """[8, SEQ, IN_DIM] f32 -> X^T bf16 [128, (plane, k, t, b)] hi/lo planes"""
    a = x_core.reshape(BPC, SEQ, NK, 128).transpose(3, 2, 1, 0)  # [p, k, t, b]
    a = np.ascontiguousarray(a.reshape(128, NK * SEQ * BPC))
    hi = a.astype(BF)
    if not gate3:
        return hi
    lo = (a - hi.astype(np.float32)).astype(BF)
    return np.concatenate([hi, lo], axis=1)


def _pack_w(w, nk, split=False):
    """[HID, D] -> lhsT tiles [128, (plane, k, q, 128)]: w[q*128+j, k*128+p] at [p, k,q,j]"""
    a = w.reshape(NQ, 128, nk, 128).transpose(3, 2, 0, 1)  # [p, k, q, j]
    a = np.ascontiguousarray(a.reshape(128, nk * NQ * 128))
    hi = a.astype(BF)
    if not split:
        return hi
    lo = (a - hi.astype(np.float32)).astype(BF)
    return np.concatenate([hi, lo], axis=1)


def _pack_bias(b):
    return np.ascontiguousarray(b.reshape(NQ, 128).T).astype(np.float32)


def _pack_h0(h0_core):
    a = h0_core.reshape(BPC, NQ, 128).transpose(2, 1, 0)
    return np.ascontiguousarray(a.reshape(128, SW)).astype(BF)


def _unpack_out(outc):
    """[128, SEQ*32] f32 -> [8, SEQ, HID] f32"""
    a = np.asarray(outc, dtype=np.float32).reshape(128, SEQ, NQ, BPC).transpose(3, 1, 2, 0)
    return np.ascontiguousarray(a.reshape(BPC, SEQ, HID))


def make_in_maps(inputs, h0, weight_ih, weight_hh, bias_ih, bias_hh, gate3=True):
    inputs = np.asarray(inputs, dtype=np.float32)
    h0 = np.asarray(h0, dtype=np.float32)
    bias = np.asarray(bias_ih, dtype=np.float32) + np.asarray(bias_hh, dtype=np.float32)
    wih_p = _pack_w(np.asarray(weight_ih, dtype=np.float32), NK, split=gate3)
    whh_p = _pack_w(np.asarray(weight_hh, dtype=np.float32), NQ)
    bias_p = _pack_bias(bias)
    in_maps = []
    for i in range(NCORES):
        bs = slice(i * BPC, (i + 1) * BPC)
        in_maps.append({
            "xt": _pack_xt(inputs[bs], gate3=gate3),
            "wih": wih_p,
            "whh": whh_p,
            "bias": bias_p,
            "h0t": _pack_h0(h0[0, bs]),
        })
    return in_maps


_CACHE = {}


def kernel(inputs, h0, weight_ih, weight_hh, bias_ih, bias_hh):
    in_maps = make_in_maps(inputs, h0, weight_ih, weight_hh, bias_ih, bias_hh)
    if "nc" not in _CACHE:
        _CACHE["nc"] = build_nc()
    res = run_bass_kernel_spmd(_CACHE["nc"], in_maps, core_ids=list(range(NCORES)))
    outputs = np.empty((BATCH, SEQ, HID), np.float32)
    for i in range(NCORES):
        outputs[i * BPC:(i + 1) * BPC] = _unpack_out(res.results[i]["outc"])
    h_n = np.ascontiguousarray(outputs[:, -1, :][None])
    return outputs, h_n
